# revision 3
# baseline (speedup 1.0000x reference)
"""Embedding-bag kernel for Trainium2, 8 NeuronCores — v8.

Design (v4-final + c0-rank dealing):
- Data-parallel: batch rows are dealt to the 8 cores by global chunk-0
  token-count rank (core c takes rows ranked c, c+8, ...), per table, so
  all cores see near-identical per-block column maxima and the SPMD-shared
  K padding collapses (~5% fewer gather indices). Outputs un-permuted on
  the host.
- Both embedding tables replicated per core in DRAM as one concatenated
  tensor (2 tables x 2 vocab chunks of 50001 rows; chunks keep indices in
  int16 range via a +32768 base shift).
- Per (table, 128-row block, chunk): one SWDGE dma_gather job split into
  pieces of <=63 columns (8064 idxs = 505 ring descriptors; 2 fit a
  queue's 1024-desc ring).
- single_packet=False: coalesced packets cap at 64 descriptors (hang
  beyond 8-column gathers); per-descriptor packets also let the 16 SDMA
  engines interleave queues (17.5ns/desc vs 26.3).
- Pieces rotate across all 4 SWDGE queues: the Q7 complex executes one
  gather per queue concurrently (~4-way); per-gather latency is
  ~1.3us + 6.7ns/idx. Single-queue schedules serialize (2.03ms).
- Vector engine reduces each job by contiguous halving adds to <=8
  columns, then one strided tensor_reduce; slot recycling is signalled by
  then_inc on the final reader.

Measured limits (2026-08 session; probes in probe.py/probe2.py, variants in
kernel_v9/v10/v11.py — all slower or equal):
- The bottleneck is SWDGE descriptor GENERATION on the Q7 cpu pairs:
  ~7.8ns/idx per queue (63us per 8064-idx gather), one cpu pair per queue,
  4 queues max (ucode MAX_SWDGE_QUEUES). The 4 pairs generate concurrently
  (the engine instruction parks only while its queue's pair is busy), but
  contention caps the aggregate at ~2.4-2.5ns/idx = ~500-530us for the
  ~219k idxs/core. This kernel sits at that wall; HW time varies
  521-555us run-to-run (~±6%).
- DMA drain is NOT the bottleneck: bursts hit ~220GB/s (the 256B-random
  cap; 16 engines x 4 queue rings) while generation feeds ~110GB/s.
- Dead ends measured: prepare_only+trigger_dma serializes generation on
  the engine (1.78ms); transpose-mode gathers generate at the same rate
  (tx cpu still pushes 1 desc/idx); SBUF-source gather 2.58ns/idx (no
  better); single_packet=True at <=8 cols 2.8ns/idx; 32-col pieces equal
  63-col; 8 per-piece slots + ramp (v10) 655us; tail-split (v11) 555us.
- Vector tree reduce ~350us busy under DMA contention (66-142 G elem/s),
  hidden under the gather wall; strided direct tensor_reduce is equal.
- PE/dense-count-matmul hybrids die on building/loading the count matrix
  (dense C is 50-100x the index bytes; bit-packed needs >100us DVE expand).
- Round 2: time budget decomposes as ~428us pure pair generation
  (219k idx x 7.8ns / 4 pairs, pairs ~95% busy in the trace) + ~22us head
  (ucode cold start + gidx DMA) + ~50us tail (last gen+drain, final trees,
  out DMA, ~8us Block teardown). Scheduling variants that targeted the
  residue all landed inside run noise: size-sorted job order (v12, 530us),
  per-piece trees consuming each gather as it lands (v13, 527us),
  tail-split pieces (v11, 555us). This v8 schedule stays.
"""

import sys

if "/opt/trn_rl_repo" not in sys.path:
    sys.path.insert(0, "/opt/trn_rl_repo")

from contextlib import ExitStack

import numpy as np

import concourse.bacc as bacc
import concourse.bass as bass
import concourse.mybir as mybir
from concourse import library_config
from concourse.bass_utils import run_bass_kernel_spmd

N_CORES = 8
P = 128
VOCAB = 100000
SEQ = 200
DIM = 64
BATCH = 4096

N_CHUNKS = 2
CHUNK = VOCAB // N_CHUNKS          # 50000 (signed int16 indexing)
CHUNK_ROWS = CHUNK + 1             # + zero pad row
BASE_SHIFT = 32768                 # in_ap base shifted this many rows in
PAD_IDX = CHUNK - BASE_SHIFT       # local index of the zero row (positive)
GMAX_COLS = 63                     # 8064 idxs -> 505 descs/dma; 2 fit a ring
NBUF = 4
NQ = 4


def _gather_plan(kj):
    """Split kj columns into balanced pieces of <=GMAX_COLS."""
    n = -(-kj // GMAX_COLS)
    base = kj // n
    rem = kj % n
    return [base + (1 if i < rem else 0) for i in range(n)]


def build_nc(K, n_blocks, idx_cols, split_col):
    """K: [2, N_CHUNKS, n_blocks] exact max counts (identical across cores).
    idx_cols: total int16 columns of gidx. split_col: boundary of the first
    idx DMA (gathers whose columns start past it wait for the second DMA)."""
    kmax = int(K.max())

    nc = bacc.Bacc("TRN2", debug=False, num_swdge_queues=NQ)

    emb_cat = nc.dram_tensor(
        "emb_cat", [2 * N_CHUNKS * CHUNK_ROWS, DIM], mybir.dt.float32,
        kind="ExternalInput",
    )
    gidx = nc.dram_tensor("gidx", [P, idx_cols], mybir.dt.int16, kind="ExternalInput")
    out_pri = nc.dram_tensor("out_pri", [n_blocks * P, DIM], mybir.dt.float32, kind="ExternalOutput")
    out_sec = nc.dram_tensor("out_sec", [n_blocks * P, DIM], mybir.dt.float32, kind="ExternalOutput")
    outs = (out_pri, out_sec)

    jobs = [(t, b, k) for t in range(2) for b in range(n_blocks) for k in range(N_CHUNKS)]

    with (
        nc.Block() as _block,
        nc.sbuf_tensor("gidx_sb", [P, idx_cols], mybir.dt.int16) as gidx_sb,
        nc.semaphore("io") as io,
        ExitStack() as stack,
    ):
        slots = [
            stack.enter_context(
                nc.sbuf_tensor(f"slot{i}", [P, kmax * DIM], mybir.dt.float32)
            )
            for i in range(NBUF)
        ]
        accs = [
            stack.enter_context(
                nc.sbuf_tensor(f"acc{t}_{b}", [P, DIM], mybir.dt.float32)
            )
            for t in range(2)
            for b in range(n_blocks)
        ]
        tmp = stack.enter_context(nc.sbuf_tensor("tmp", [P, DIM], mybir.dt.float32))
        done = [
            [stack.enter_context(nc.semaphore(f"done{i}_{q}")) for q in range(NQ)]
            for i in range(NBUF)
        ]
        free = [stack.enter_context(nc.semaphore(f"free{i}")) for i in range(NBUF)]
        oready = stack.enter_context(nc.semaphore("oready"))

        # ---- sync engine: two-stage index load so gathers start early
        nc.sync.dma_start(gidx_sb[:, :split_col], gidx[:, :split_col]).then_inc(io, 16)
        nc.sync.dma_start(gidx_sb[:, split_col:], gidx[:, split_col:]).then_inc(io, 16)

        # ---- gpsimd: all gathers
        nc.gpsimd.load_library(library_config.mlp)
        nc.gpsimd.wait_ge(io, 16)
        waited_full = False
        gq = 0            # queue rotation counter
        icol = 0          # running int16 column offset into gidx_sb
        done_target = [[0] * NQ for _ in range(NBUF)]
        for j, (t, b, k) in enumerate(jobs):
            slot = j % NBUF
            if j >= NBUF:
                nc.gpsimd.wait_ge(free[slot], j // NBUF)
            kj = int(K[t, k, b])
            base = (t * N_CHUNKS + k) * CHUNK_ROWS + BASE_SHIFT
            src = emb_cat[base:(t * N_CHUNKS + k + 1) * CHUNK_ROWS, :]
            g3 = slots[slot][:].rearrange("p (c d) -> p c d", d=DIM)
            col = 0
            for size in _gather_plan(kj):
                nidx = size * P
                ic = nidx // 16
                if not waited_full and icol + ic > split_col:
                    nc.gpsimd.wait_ge(io, 32)
                    waited_full = True
                q = gq % NQ
                nc.gpsimd.dma_gather(
                    g3[:, col:col + size, :],
                    src,
                    gidx_sb[:, icol:icol + ic],
                    nidx,
                    nidx,
                    DIM,
                    queue_num=q,
                    single_packet=False,
                ).then_inc(done[slot][q], 16)
                done_target[slot][q] += 16
                gq += 1
                icol += ic
                col += size
            jobs[j] = (t, b, k, slot, tuple(done_target[slot]), kj)

        # ---- vector: halving-tree reduce, accumulate chunks, recycle slots
        for j, (t, b, k, slot, tgts, kj) in enumerate(jobs):
            for q in range(NQ):
                if tgts[q]:
                    nc.vector.wait_ge(done[slot][q], tgts[q])
            g = slots[slot]
            n = kj
            while n > 8:
                h = n // 2
                nc.vector.tensor_add(
                    out=g[:, : h * DIM],
                    in0=g[:, : h * DIM],
                    in1=g[:, (n - h) * DIM : n * DIM],
                )
                n -= h
            gv = g[:].rearrange("p (c d) -> p d c", d=DIM)[:, :, :n]
            acc = accs[t * n_blocks + b]
            red_out = acc if k == 0 else tmp
            red = nc.vector.tensor_reduce(
                out=red_out[:], in_=gv, axis=mybir.AxisListType.X,
                op=mybir.AluOpType.add,
            )
            # the reduce is the last reader of the slot
            red.then_inc(free[slot], 1)
            if k == 0:
                continue
            nc.vector.tensor_add(out=acc[:], in0=acc[:], in1=tmp[:]).then_inc(
                oready, 1
            )

        # ---- sync engine: write outputs as accs complete
        m = 0
        for t in range(2):
            for b in range(n_blocks):
                m += 1
                nc.sync.wait_ge(oready, m)
                nc.sync.dma_start(
                    out=outs[t][b * P:(b + 1) * P, :],
                    in_=accs[t * n_blocks + b][:],
                ).then_inc(io, 16)
        nc.sync.wait_ge(io, 32 + m * 16)

    nc.compile()
    return nc


def _pack_core(idx_sorted, K, n_blocks):
    """idx_sorted: [2][bc, SEQ] row-sorted core indices. Returns gidx."""
    streams = []
    for t in range(2):
        for b in range(n_blocks):
            rows = idx_sorted[t][b * P:(b + 1) * P]
            for k in range(N_CHUNKS):
                kj = int(K[t, k, b])
                mask = (rows // CHUNK) == k
                local = (rows - k * CHUNK - BASE_SHIFT).astype(np.int64)
                order = np.argsort(~mask, axis=1, kind="stable")
                sortloc = np.take_along_axis(local, order, axis=1)
                cnt = mask.sum(axis=1)
                pad_cols = max(kj - SEQ, 0)
                if pad_cols:
                    sortloc = np.concatenate(
                        [sortloc, np.zeros((P, pad_cols), np.int64)], axis=1
                    )
                sel = sortloc[:, :kj]
                sel = np.where(np.arange(kj)[None, :] < cnt[:, None], sel, PAD_IDX)
                # Every gather's final stream slot (lane 127, last column of
                # the gather) must be >= 0: ucode trims trailing negatives.
                row127 = sel[127].copy()
                lasts = []
                c = 0
                for size in _gather_plan(kj):
                    c += size
                    lasts.append(c - 1)
                lastset = set(lasts)
                for last in lasts:
                    if row127[last] < 0:
                        cand = [jj for jj in range(kj)
                                if row127[jj] >= 0 and jj not in lastset]
                        assert cand, "no non-negative index for lane 127"
                        jj = cand[0]
                        row127[last], row127[jj] = row127[jj], row127[last]
                sel[127] = row127
                # column-major stream, split per gather
                c = 0
                for size in _gather_plan(kj):
                    streams.append(sel[:, c:c + size].T.ravel())
                    c += size
    s = np.concatenate(streams).astype(np.int16)
    wrapped = s.reshape(-1, 16).T
    return np.tile(wrapped, (8, 1)).copy()


def kernel(inputs_pri, inputs_sec, emb_pri, emb_sec, _trace=False, _trace_kwargs=None):
    inputs_pri = np.ascontiguousarray(np.asarray(inputs_pri, dtype=np.int32))
    inputs_sec = np.ascontiguousarray(np.asarray(inputs_sec, dtype=np.int32))
    emb_pri = np.ascontiguousarray(np.asarray(emb_pri, dtype=np.float32))
    emb_sec = np.ascontiguousarray(np.asarray(emb_sec, dtype=np.float32))

    batch = inputs_pri.shape[0]
    bc = batch // N_CORES
    n_blocks = bc // P

    emb_cat = np.zeros((2, N_CHUNKS, CHUNK_ROWS, DIM), np.float32)
    for t, emb in enumerate((emb_pri, emb_sec)):
        for k in range(N_CHUNKS):
            emb_cat[t, k, :CHUNK] = emb[k * CHUNK:(k + 1) * CHUNK]
    emb_cat = np.ascontiguousarray(emb_cat.reshape(2 * N_CHUNKS * CHUNK_ROWS, DIM))

    # Deal rows to cores by global chunk-0-count rank (per table): core c
    # takes rows ranked c, c+8, ... so all cores share near-identical block
    # profiles and the SPMD-shared per-block maxima K carry ~no cross-core
    # padding. Each core's dealt rows are already c0-sorted by construction.
    deal = []     # per core: [2][bc] global row ids, c0-ascending
    K = np.zeros((2, N_CHUNKS, n_blocks), np.int64)
    for t, full in enumerate((inputs_pri, inputs_sec)):
        c0_all = ((full // CHUNK) == 0).sum(axis=1)
        rank = np.argsort(c0_all, kind="stable")
        for c in range(N_CORES):
            ids = rank[c::N_CORES]
            if t == 0:
                deal.append([ids])
            else:
                deal[c].append(ids)
            c0s = c0_all[ids]
            for b in range(n_blocks):
                blk = c0s[b * P:(b + 1) * P]
                K[t, 0, b] = max(K[t, 0, b], blk.max())
                K[t, 1, b] = max(K[t, 1, b], SEQ - blk.min())
    K = np.maximum(K, 1)

    total_cols = int(K.sum())
    idx_cols = total_cols * P // 16
    # first DMA covers just job 0's columns so gathers start ASAP; the second
    # (bulk) DMA overlaps job 0's gathers
    split_col = max(64, -(-(int(K[0, 0, 0]) * P // 16) // 64) * 64 + 64)
    split_col = min(split_col, idx_cols - 64)

    nc = build_nc(K, n_blocks, idx_cols, split_col)

    in_maps = []
    for c in range(N_CORES):
        rows_c = [inputs_pri[deal[c][0]], inputs_sec[deal[c][1]]]
        gidx = _pack_core(rows_c, K, n_blocks)
        assert gidx.shape[1] == idx_cols
        in_maps.append({"emb_cat": emb_cat, "gidx": gidx})

    kwargs = {}
    if _trace:
        kwargs["trace"] = True
        if _trace_kwargs:
            kwargs.update(_trace_kwargs)
    res = run_bass_kernel_spmd(nc, in_maps, list(range(N_CORES)), **kwargs)
    outs = res.results
    out_pri = np.empty((batch, DIM), np.float32)
    out_sec = np.empty((batch, DIM), np.float32)
    for c in range(N_CORES):
        for t, out_full in enumerate((out_pri, out_sec)):
            res_c = outs[c]["out_pri" if t == 0 else "out_sec"]
            out_full[deal[c][t]] = res_c
    if _trace:
        return (out_pri, out_sec), res
    return out_pri, out_sec



# revision 4
# speedup vs baseline: 6.7180x; 6.7180x over previous
"""Embedding-bag kernel for Trainium2, 8 NeuronCores — v8.

Design (v4-final + c0-rank dealing):
- Data-parallel: batch rows are dealt to the 8 cores by global chunk-0
  token-count rank (core c takes rows ranked c, c+8, ...), per table, so
  all cores see near-identical per-block column maxima and the SPMD-shared
  K padding collapses (~5% fewer gather indices). Outputs un-permuted on
  the host.
- Both embedding tables replicated per core in DRAM as one concatenated
  tensor (2 tables x 2 vocab chunks of 50001 rows; chunks keep indices in
  int16 range via a +32768 base shift).
- Per (table, 128-row block, chunk): one SWDGE dma_gather job split into
  pieces of <=63 columns (8064 idxs = 505 ring descriptors; 2 fit a
  queue's 1024-desc ring).
- single_packet=False: coalesced packets cap at 64 descriptors (hang
  beyond 8-column gathers); per-descriptor packets also let the 16 SDMA
  engines interleave queues (17.5ns/desc vs 26.3).
- Pieces rotate across all 4 SWDGE queues: the Q7 complex executes one
  gather per queue concurrently (~4-way); per-gather latency is
  ~1.3us + 6.7ns/idx. Single-queue schedules serialize (2.03ms).
- Vector engine reduces each job by contiguous halving adds to <=8
  columns, then one strided tensor_reduce; slot recycling is signalled by
  then_inc on the final reader.

Measured limits (2026-08 session; probes in probe.py/probe2.py, variants in
kernel_v9/v10/v11.py — all slower or equal):
- The bottleneck is SWDGE descriptor GENERATION on the Q7 cpu pairs:
  ~7.8ns/idx per queue (63us per 8064-idx gather), one cpu pair per queue,
  4 queues max (ucode MAX_SWDGE_QUEUES). The 4 pairs generate concurrently
  (the engine instruction parks only while its queue's pair is busy), but
  contention caps the aggregate at ~2.4-2.5ns/idx = ~500-530us for the
  ~219k idxs/core. This kernel sits at that wall; HW time varies
  521-555us run-to-run (~±6%).
- DMA drain is NOT the bottleneck: bursts hit ~220GB/s (the 256B-random
  cap; 16 engines x 4 queue rings) while generation feeds ~110GB/s.
- Dead ends measured: prepare_only+trigger_dma serializes generation on
  the engine (1.78ms); transpose-mode gathers generate at the same rate
  (tx cpu still pushes 1 desc/idx); SBUF-source gather 2.58ns/idx (no
  better); single_packet=True at <=8 cols 2.8ns/idx; 32-col pieces equal
  63-col; 8 per-piece slots + ramp (v10) 655us; tail-split (v11) 555us.
- Vector tree reduce ~350us busy under DMA contention (66-142 G elem/s),
  hidden under the gather wall; strided direct tensor_reduce is equal.
- PE/dense-count-matmul hybrids die on building/loading the count matrix
  (dense C is 50-100x the index bytes; bit-packed needs >100us DVE expand).
- Round 2: time budget decomposes as ~428us pure pair generation
  (219k idx x 7.8ns / 4 pairs, pairs ~95% busy in the trace) + ~22us head
  (ucode cold start + gidx DMA) + ~50us tail (last gen+drain, final trees,
  out DMA, ~8us Block teardown). Scheduling variants that targeted the
  residue all landed inside run noise: size-sorted job order (v12, 530us),
  per-piece trees consuming each gather as it lands (v13, 527us),
  tail-split pieces (v11, 555us). This v8 schedule stays.
- Round 3: ap_gather (Q7 SBUF gather, probe3.py) measured ~27ns/idx
  (~221us per 8192-idx call) — 10x worse than dma_gather; not a viable
  alternative path. Ucode libraries load one-at-a-time (PseudoReload-
  LibraryIndex), so dma_gather + ap_gather could not overlap anyway
  without building a custom combined library.
"""

import sys

if "/opt/trn_rl_repo" not in sys.path:
    sys.path.insert(0, "/opt/trn_rl_repo")

from contextlib import ExitStack

import numpy as np

import concourse.bacc as bacc
import concourse.bass as bass
import concourse.mybir as mybir
from concourse import library_config
from concourse.bass_utils import run_bass_kernel_spmd

N_CORES = 8
P = 128
VOCAB = 100000
SEQ = 200
DIM = 64
BATCH = 4096

N_CHUNKS = 2
CHUNK = VOCAB // N_CHUNKS          # 50000 (signed int16 indexing)
CHUNK_ROWS = CHUNK + 1             # + zero pad row
BASE_SHIFT = 32768                 # in_ap base shifted this many rows in
PAD_IDX = CHUNK - BASE_SHIFT       # local index of the zero row (positive)
GMAX_COLS = 63                     # 8064 idxs -> 505 descs/dma; 2 fit a ring
NBUF = 4
NQ = 4


def _gather_plan(kj):
    """Split kj columns into balanced pieces of <=GMAX_COLS."""
    n = -(-kj // GMAX_COLS)
    base = kj // n
    rem = kj % n
    return [base + (1 if i < rem else 0) for i in range(n)]


def build_nc(K, n_blocks, idx_cols, split_col):
    """K: [2, N_CHUNKS, n_blocks] exact max counts (identical across cores).
    idx_cols: total int16 columns of gidx. split_col: boundary of the first
    idx DMA (gathers whose columns start past it wait for the second DMA)."""
    kmax = int(K.max())

    nc = bacc.Bacc("TRN2", debug=False, num_swdge_queues=NQ)

    emb_cat = nc.dram_tensor(
        "emb_cat", [2 * N_CHUNKS * CHUNK_ROWS, DIM], mybir.dt.float32,
        kind="ExternalInput",
    )
    gidx = nc.dram_tensor("gidx", [P, idx_cols], mybir.dt.int16, kind="ExternalInput")
    out_pri = nc.dram_tensor("out_pri", [n_blocks * P, DIM], mybir.dt.float32, kind="ExternalOutput")
    out_sec = nc.dram_tensor("out_sec", [n_blocks * P, DIM], mybir.dt.float32, kind="ExternalOutput")
    outs = (out_pri, out_sec)

    jobs = [(t, b, k) for t in range(2) for b in range(n_blocks) for k in range(N_CHUNKS)]

    with (
        nc.Block() as _block,
        nc.sbuf_tensor("gidx_sb", [P, idx_cols], mybir.dt.int16) as gidx_sb,
        nc.semaphore("io") as io,
        ExitStack() as stack,
    ):
        slots = [
            stack.enter_context(
                nc.sbuf_tensor(f"slot{i}", [P, kmax * DIM], mybir.dt.float32)
            )
            for i in range(NBUF)
        ]
        accs = [
            stack.enter_context(
                nc.sbuf_tensor(f"acc{t}_{b}", [P, DIM], mybir.dt.float32)
            )
            for t in range(2)
            for b in range(n_blocks)
        ]
        tmp = stack.enter_context(nc.sbuf_tensor("tmp", [P, DIM], mybir.dt.float32))
        done = [
            [stack.enter_context(nc.semaphore(f"done{i}_{q}")) for q in range(NQ)]
            for i in range(NBUF)
        ]
        free = [stack.enter_context(nc.semaphore(f"free{i}")) for i in range(NBUF)]
        oready = stack.enter_context(nc.semaphore("oready"))

        # ---- sync engine: two-stage index load so gathers start early
        nc.sync.dma_start(gidx_sb[:, :split_col], gidx[:, :split_col]).then_inc(io, 16)
        nc.sync.dma_start(gidx_sb[:, split_col:], gidx[:, split_col:]).then_inc(io, 16)

        # ---- gpsimd: all gathers
        nc.gpsimd.load_library(library_config.mlp)
        nc.gpsimd.wait_ge(io, 16)
        waited_full = False
        gq = 0            # queue rotation counter
        icol = 0          # running int16 column offset into gidx_sb
        done_target = [[0] * NQ for _ in range(NBUF)]
        for j, (t, b, k) in enumerate(jobs):
            slot = j % NBUF
            if j >= NBUF:
                nc.gpsimd.wait_ge(free[slot], j // NBUF)
            kj = int(K[t, k, b])
            base = (t * N_CHUNKS + k) * CHUNK_ROWS + BASE_SHIFT
            src = emb_cat[base:(t * N_CHUNKS + k + 1) * CHUNK_ROWS, :]
            g3 = slots[slot][:].rearrange("p (c d) -> p c d", d=DIM)
            col = 0
            for size in _gather_plan(kj):
                nidx = size * P
                ic = nidx // 16
                if not waited_full and icol + ic > split_col:
                    nc.gpsimd.wait_ge(io, 32)
                    waited_full = True
                q = gq % NQ
                nc.gpsimd.dma_gather(
                    g3[:, col:col + size, :],
                    src,
                    gidx_sb[:, icol:icol + ic],
                    nidx,
                    nidx,
                    DIM,
                    queue_num=q,
                    single_packet=False,
                ).then_inc(done[slot][q], 16)
                done_target[slot][q] += 16
                gq += 1
                icol += ic
                col += size
            jobs[j] = (t, b, k, slot, tuple(done_target[slot]), kj)

        # ---- vector: halving-tree reduce, accumulate chunks, recycle slots
        for j, (t, b, k, slot, tgts, kj) in enumerate(jobs):
            for q in range(NQ):
                if tgts[q]:
                    nc.vector.wait_ge(done[slot][q], tgts[q])
            g = slots[slot]
            n = kj
            while n > 8:
                h = n // 2
                nc.vector.tensor_add(
                    out=g[:, : h * DIM],
                    in0=g[:, : h * DIM],
                    in1=g[:, (n - h) * DIM : n * DIM],
                )
                n -= h
            gv = g[:].rearrange("p (c d) -> p d c", d=DIM)[:, :, :n]
            acc = accs[t * n_blocks + b]
            red_out = acc if k == 0 else tmp
            red = nc.vector.tensor_reduce(
                out=red_out[:], in_=gv, axis=mybir.AxisListType.X,
                op=mybir.AluOpType.add,
            )
            # the reduce is the last reader of the slot
            red.then_inc(free[slot], 1)
            if k == 0:
                continue
            nc.vector.tensor_add(out=acc[:], in0=acc[:], in1=tmp[:]).then_inc(
                oready, 1
            )

        # ---- sync engine: write outputs as accs complete
        m = 0
        for t in range(2):
            for b in range(n_blocks):
                m += 1
                nc.sync.wait_ge(oready, m)
                nc.sync.dma_start(
                    out=outs[t][b * P:(b + 1) * P, :],
                    in_=accs[t * n_blocks + b][:],
                ).then_inc(io, 16)
        nc.sync.wait_ge(io, 32 + m * 16)

    nc.compile()
    return nc


def _pack_core(idx_sorted, K, n_blocks):
    """idx_sorted: [2][bc, SEQ] row-sorted core indices. Returns gidx."""
    streams = []
    for t in range(2):
        for b in range(n_blocks):
            rows = idx_sorted[t][b * P:(b + 1) * P]
            for k in range(N_CHUNKS):
                kj = int(K[t, k, b])
                mask = (rows // CHUNK) == k
                local = (rows - k * CHUNK - BASE_SHIFT).astype(np.int64)
                order = np.argsort(~mask, axis=1, kind="stable")
                sortloc = np.take_along_axis(local, order, axis=1)
                cnt = mask.sum(axis=1)
                pad_cols = max(kj - SEQ, 0)
                if pad_cols:
                    sortloc = np.concatenate(
                        [sortloc, np.zeros((P, pad_cols), np.int64)], axis=1
                    )
                sel = sortloc[:, :kj]
                sel = np.where(np.arange(kj)[None, :] < cnt[:, None], sel, PAD_IDX)
                # Every gather's final stream slot (lane 127, last column of
                # the gather) must be >= 0: ucode trims trailing negatives.
                row127 = sel[127].copy()
                lasts = []
                c = 0
                for size in _gather_plan(kj):
                    c += size
                    lasts.append(c - 1)
                lastset = set(lasts)
                for last in lasts:
                    if row127[last] < 0:
                        cand = [jj for jj in range(kj)
                                if row127[jj] >= 0 and jj not in lastset]
                        assert cand, "no non-negative index for lane 127"
                        jj = cand[0]
                        row127[last], row127[jj] = row127[jj], row127[last]
                sel[127] = row127
                # column-major stream, split per gather
                c = 0
                for size in _gather_plan(kj):
                    streams.append(sel[:, c:c + size].T.ravel())
                    c += size
    s = np.concatenate(streams).astype(np.int16)
    wrapped = s.reshape(-1, 16).T
    return np.tile(wrapped, (8, 1)).copy()


def kernel(inputs_pri, inputs_sec, emb_pri, emb_sec, _trace=False, _trace_kwargs=None):
    inputs_pri = np.ascontiguousarray(np.asarray(inputs_pri, dtype=np.int32))
    inputs_sec = np.ascontiguousarray(np.asarray(inputs_sec, dtype=np.int32))
    emb_pri = np.ascontiguousarray(np.asarray(emb_pri, dtype=np.float32))
    emb_sec = np.ascontiguousarray(np.asarray(emb_sec, dtype=np.float32))

    batch = inputs_pri.shape[0]
    bc = batch // N_CORES
    n_blocks = bc // P

    emb_cat = np.zeros((2, N_CHUNKS, CHUNK_ROWS, DIM), np.float32)
    for t, emb in enumerate((emb_pri, emb_sec)):
        for k in range(N_CHUNKS):
            emb_cat[t, k, :CHUNK] = emb[k * CHUNK:(k + 1) * CHUNK]
    emb_cat = np.ascontiguousarray(emb_cat.reshape(2 * N_CHUNKS * CHUNK_ROWS, DIM))

    # Deal rows to cores by global chunk-0-count rank (per table): core c
    # takes rows ranked c, c+8, ... so all cores share near-identical block
    # profiles and the SPMD-shared per-block maxima K carry ~no cross-core
    # padding. Each core's dealt rows are already c0-sorted by construction.
    deal = []     # per core: [2][bc] global row ids, c0-ascending
    K = np.zeros((2, N_CHUNKS, n_blocks), np.int64)
    for t, full in enumerate((inputs_pri, inputs_sec)):
        c0_all = ((full // CHUNK) == 0).sum(axis=1)
        rank = np.argsort(c0_all, kind="stable")
        for c in range(N_CORES):
            ids = rank[c::N_CORES]
            if t == 0:
                deal.append([ids])
            else:
                deal[c].append(ids)
            c0s = c0_all[ids]
            for b in range(n_blocks):
                blk = c0s[b * P:(b + 1) * P]
                K[t, 0, b] = max(K[t, 0, b], blk.max())
                K[t, 1, b] = max(K[t, 1, b], SEQ - blk.min())
    K = np.maximum(K, 1)

    total_cols = int(K.sum())
    idx_cols = total_cols * P // 16
    # first DMA covers just job 0's columns so gathers start ASAP; the second
    # (bulk) DMA overlaps job 0's gathers
    split_col = max(64, -(-(int(K[0, 0, 0]) * P // 16) // 64) * 64 + 64)
    split_col = min(split_col, idx_cols - 64)

    nc = build_nc(K, n_blocks, idx_cols, split_col)

    in_maps = []
    for c in range(N_CORES):
        rows_c = [inputs_pri[deal[c][0]], inputs_sec[deal[c][1]]]
        gidx = _pack_core(rows_c, K, n_blocks)
        assert gidx.shape[1] == idx_cols
        in_maps.append({"emb_cat": emb_cat, "gidx": gidx})

    kwargs = {}
    if _trace:
        kwargs["trace"] = True
        if _trace_kwargs:
            kwargs.update(_trace_kwargs)
    res = run_bass_kernel_spmd(nc, in_maps, list(range(N_CORES)), **kwargs)
    outs = res.results
    out_pri = np.empty((batch, DIM), np.float32)
    out_sec = np.empty((batch, DIM), np.float32)
    for c in range(N_CORES):
        for t, out_full in enumerate((out_pri, out_sec)):
            res_c = outs[c]["out_pri" if t == 0 else "out_sec"]
            out_full[deal[c][t]] = res_c
    if _trace:
        return (out_pri, out_sec), res
    return out_pri, out_sec



# revision 5
# speedup vs baseline: 6.9521x; 1.0349x over previous
"""Embedding-bag kernel for Trainium2, 8 NeuronCores — v14 (v8 + Q7 warmup overlap).

Design (v4-final + c0-rank dealing):
- Data-parallel: batch rows are dealt to the 8 cores by global chunk-0
  token-count rank (core c takes rows ranked c, c+8, ...), per table, so
  all cores see near-identical per-block column maxima and the SPMD-shared
  K padding collapses (~5% fewer gather indices). Outputs un-permuted on
  the host.
- Both embedding tables replicated per core in DRAM as one concatenated
  tensor (2 tables x 2 vocab chunks of 50001 rows; chunks keep indices in
  int16 range via a +32768 base shift).
- Per (table, 128-row block, chunk): one SWDGE dma_gather job split into
  pieces of <=63 columns (8064 idxs = 505 ring descriptors; 2 fit a
  queue's 1024-desc ring).
- single_packet=False: coalesced packets cap at 64 descriptors (hang
  beyond 8-column gathers); per-descriptor packets also let the 16 SDMA
  engines interleave queues (17.5ns/desc vs 26.3).
- Pieces rotate across all 4 SWDGE queues: the Q7 complex executes one
  gather per queue concurrently (~4-way); per-gather latency is
  ~1.3us + 6.7ns/idx. Single-queue schedules serialize (2.03ms).
- Vector engine reduces each job by contiguous halving adds to <=8
  columns, then one strided tensor_reduce; slot recycling is signalled by
  then_inc on the final reader.

Measured limits (2026-08 session; probes in probe.py/probe2.py, variants in
kernel_v9/v10/v11.py — all slower or equal):
- The bottleneck is SWDGE descriptor GENERATION on the Q7 cpu pairs:
  ~7.8ns/idx per queue (63us per 8064-idx gather), one cpu pair per queue,
  4 queues max (ucode MAX_SWDGE_QUEUES). The 4 pairs generate concurrently
  (the engine instruction parks only while its queue's pair is busy), but
  contention caps the aggregate at ~2.4-2.5ns/idx = ~500-530us for the
  ~219k idxs/core. This kernel sits at that wall; HW time varies
  521-555us run-to-run (~±6%).
- DMA drain is NOT the bottleneck: bursts hit ~220GB/s (the 256B-random
  cap; 16 engines x 4 queue rings) while generation feeds ~110GB/s.
- Dead ends measured: prepare_only+trigger_dma serializes generation on
  the engine (1.78ms); transpose-mode gathers generate at the same rate
  (tx cpu still pushes 1 desc/idx); SBUF-source gather 2.58ns/idx (no
  better); single_packet=True at <=8 cols 2.8ns/idx; 32-col pieces equal
  63-col; 8 per-piece slots + ramp (v10) 655us; tail-split (v11) 555us.
- Vector tree reduce ~350us busy under DMA contention (66-142 G elem/s),
  hidden under the gather wall; strided direct tensor_reduce is equal.
- PE/dense-count-matmul hybrids die on building/loading the count matrix
  (dense C is 50-100x the index bytes; bit-packed needs >100us DVE expand).
- Round 2: time budget decomposes as ~428us pure pair generation
  (219k idx x 7.8ns / 4 pairs, pairs ~95% busy in the trace) + ~22us head
  (ucode cold start + gidx DMA) + ~50us tail (last gen+drain, final trees,
  out DMA, ~8us Block teardown). Scheduling variants that targeted the
  residue all landed inside run noise: size-sorted job order (v12, 530us),
  per-piece trees consuming each gather as it lands (v13, 527us),
  tail-split pieces (v11, 555us). This v8 schedule stays.
- Round 3: ap_gather (Q7 SBUF gather, probe3.py) measured ~27ns/idx
  (~221us per 8192-idx call) — 10x worse than dma_gather; not a viable
  alternative path. Ucode libraries load one-at-a-time (PseudoReload-
  LibraryIndex), so dma_gather + ap_gather could not overlap anyway
  without building a custom combined library.
- Round 4 (this version): 4 dummy 128-idx warmup gathers issued before the
  gidx wait overlap the ~10us Q7 ucode cold start per pair with the index
  DMA; first idx chunk shrunk to piece 0 only. Paired same-period runs:
  v14 525.5us vs v8 567.0us.
"""

import sys

if "/opt/trn_rl_repo" not in sys.path:
    sys.path.insert(0, "/opt/trn_rl_repo")

from contextlib import ExitStack

import numpy as np

import concourse.bacc as bacc
import concourse.bass as bass
import concourse.mybir as mybir
from concourse import library_config
from concourse.bass_utils import run_bass_kernel_spmd

N_CORES = 8
P = 128
VOCAB = 100000
SEQ = 200
DIM = 64
BATCH = 4096

N_CHUNKS = 2
CHUNK = VOCAB // N_CHUNKS          # 50000 (signed int16 indexing)
CHUNK_ROWS = CHUNK + 1             # + zero pad row
BASE_SHIFT = 32768                 # in_ap base shifted this many rows in
PAD_IDX = CHUNK - BASE_SHIFT       # local index of the zero row (positive)
GMAX_COLS = 63                     # 8064 idxs -> 505 descs/dma; 2 fit a ring
NBUF = 4
NQ = 4


def _gather_plan(kj):
    """Split kj columns into balanced pieces of <=GMAX_COLS."""
    n = -(-kj // GMAX_COLS)
    base = kj // n
    rem = kj % n
    return [base + (1 if i < rem else 0) for i in range(n)]


def build_nc(K, n_blocks, idx_cols, split_col):
    """K: [2, N_CHUNKS, n_blocks] exact max counts (identical across cores).
    idx_cols: total int16 columns of gidx. split_col: boundary of the first
    idx DMA (gathers whose columns start past it wait for the second DMA)."""
    kmax = int(K.max())

    nc = bacc.Bacc("TRN2", debug=False, num_swdge_queues=NQ)

    emb_cat = nc.dram_tensor(
        "emb_cat", [2 * N_CHUNKS * CHUNK_ROWS, DIM], mybir.dt.float32,
        kind="ExternalInput",
    )
    gidx = nc.dram_tensor("gidx", [P, idx_cols], mybir.dt.int16, kind="ExternalInput")
    out_pri = nc.dram_tensor("out_pri", [n_blocks * P, DIM], mybir.dt.float32, kind="ExternalOutput")
    out_sec = nc.dram_tensor("out_sec", [n_blocks * P, DIM], mybir.dt.float32, kind="ExternalOutput")
    outs = (out_pri, out_sec)

    jobs = [(t, b, k) for t in range(2) for b in range(n_blocks) for k in range(N_CHUNKS)]

    with (
        nc.Block() as _block,
        nc.sbuf_tensor("gidx_sb", [P, idx_cols], mybir.dt.int16) as gidx_sb,
        nc.semaphore("io") as io,
        ExitStack() as stack,
    ):
        slots = [
            stack.enter_context(
                nc.sbuf_tensor(f"slot{i}", [P, kmax * DIM], mybir.dt.float32)
            )
            for i in range(NBUF)
        ]
        accs = [
            stack.enter_context(
                nc.sbuf_tensor(f"acc{t}_{b}", [P, DIM], mybir.dt.float32)
            )
            for t in range(2)
            for b in range(n_blocks)
        ]
        tmp = stack.enter_context(nc.sbuf_tensor("tmp", [P, DIM], mybir.dt.float32))
        done = [
            [stack.enter_context(nc.semaphore(f"done{i}_{q}")) for q in range(NQ)]
            for i in range(NBUF)
        ]
        free = [stack.enter_context(nc.semaphore(f"free{i}")) for i in range(NBUF)]
        oready = stack.enter_context(nc.semaphore("oready"))

        # ---- sync engine: two-stage index load so gathers start early
        nc.sync.dma_start(gidx_sb[:, :split_col], gidx[:, :split_col]).then_inc(io, 16)
        nc.sync.dma_start(gidx_sb[:, split_col:], gidx[:, split_col:]).then_inc(io, 16)

        # ---- gpsimd: warm all 4 Q7 pairs (ucode cold start ~10us) while the
        # first gidx chunk is still in flight: 128 dummy idxs (zeros) per queue
        # into a scratch slot. Their DMAs complete in a few us; nothing waits.
        nc.gpsimd.load_library(library_config.mlp)
        widx = stack.enter_context(nc.sbuf_tensor("widx", [P, 8], mybir.dt.int16))
        wslot = stack.enter_context(nc.sbuf_tensor("wslot", [P, 8 * DIM], mybir.dt.float32))
        warm = stack.enter_context(nc.semaphore("warm"))
        nc.gpsimd.memset(widx[:], 0)
        w3 = wslot[:].rearrange("p (c d) -> p c d", d=DIM)
        wsrc = emb_cat[BASE_SHIFT:CHUNK_ROWS, :]
        for q in range(NQ):
            nc.gpsimd.dma_gather(
                w3[:, :1, :], wsrc, widx[:], P, P, DIM,
                queue_num=q, single_packet=False,
            ).then_inc(warm, 16)
        nc.gpsimd.wait_ge(io, 16)
        waited_full = False
        gq = 0            # queue rotation counter
        icol = 0          # running int16 column offset into gidx_sb
        done_target = [[0] * NQ for _ in range(NBUF)]
        for j, (t, b, k) in enumerate(jobs):
            slot = j % NBUF
            if j >= NBUF:
                nc.gpsimd.wait_ge(free[slot], j // NBUF)
            kj = int(K[t, k, b])
            base = (t * N_CHUNKS + k) * CHUNK_ROWS + BASE_SHIFT
            src = emb_cat[base:(t * N_CHUNKS + k + 1) * CHUNK_ROWS, :]
            g3 = slots[slot][:].rearrange("p (c d) -> p c d", d=DIM)
            col = 0
            for size in _gather_plan(kj):
                nidx = size * P
                ic = nidx // 16
                if not waited_full and icol + ic > split_col:
                    nc.gpsimd.wait_ge(io, 32)
                    waited_full = True
                q = gq % NQ
                nc.gpsimd.dma_gather(
                    g3[:, col:col + size, :],
                    src,
                    gidx_sb[:, icol:icol + ic],
                    nidx,
                    nidx,
                    DIM,
                    queue_num=q,
                    single_packet=False,
                ).then_inc(done[slot][q], 16)
                done_target[slot][q] += 16
                gq += 1
                icol += ic
                col += size
            jobs[j] = (t, b, k, slot, tuple(done_target[slot]), kj)

        # ---- vector: halving-tree reduce, accumulate chunks, recycle slots
        for j, (t, b, k, slot, tgts, kj) in enumerate(jobs):
            for q in range(NQ):
                if tgts[q]:
                    nc.vector.wait_ge(done[slot][q], tgts[q])
            g = slots[slot]
            n = kj
            while n > 8:
                h = n // 2
                nc.vector.tensor_add(
                    out=g[:, : h * DIM],
                    in0=g[:, : h * DIM],
                    in1=g[:, (n - h) * DIM : n * DIM],
                )
                n -= h
            gv = g[:].rearrange("p (c d) -> p d c", d=DIM)[:, :, :n]
            acc = accs[t * n_blocks + b]
            red_out = acc if k == 0 else tmp
            red = nc.vector.tensor_reduce(
                out=red_out[:], in_=gv, axis=mybir.AxisListType.X,
                op=mybir.AluOpType.add,
            )
            # the reduce is the last reader of the slot
            red.then_inc(free[slot], 1)
            if k == 0:
                continue
            nc.vector.tensor_add(out=acc[:], in0=acc[:], in1=tmp[:]).then_inc(
                oready, 1
            )

        # ---- sync engine: write outputs as accs complete
        m = 0
        for t in range(2):
            for b in range(n_blocks):
                m += 1
                nc.sync.wait_ge(oready, m)
                nc.sync.dma_start(
                    out=outs[t][b * P:(b + 1) * P, :],
                    in_=accs[t * n_blocks + b][:],
                ).then_inc(io, 16)
        nc.sync.wait_ge(io, 32 + m * 16)

    nc.compile()
    return nc


def _pack_core(idx_sorted, K, n_blocks):
    """idx_sorted: [2][bc, SEQ] row-sorted core indices. Returns gidx."""
    streams = []
    for t in range(2):
        for b in range(n_blocks):
            rows = idx_sorted[t][b * P:(b + 1) * P]
            for k in range(N_CHUNKS):
                kj = int(K[t, k, b])
                mask = (rows // CHUNK) == k
                local = (rows - k * CHUNK - BASE_SHIFT).astype(np.int64)
                order = np.argsort(~mask, axis=1, kind="stable")
                sortloc = np.take_along_axis(local, order, axis=1)
                cnt = mask.sum(axis=1)
                pad_cols = max(kj - SEQ, 0)
                if pad_cols:
                    sortloc = np.concatenate(
                        [sortloc, np.zeros((P, pad_cols), np.int64)], axis=1
                    )
                sel = sortloc[:, :kj]
                sel = np.where(np.arange(kj)[None, :] < cnt[:, None], sel, PAD_IDX)
                # Every gather's final stream slot (lane 127, last column of
                # the gather) must be >= 0: ucode trims trailing negatives.
                row127 = sel[127].copy()
                lasts = []
                c = 0
                for size in _gather_plan(kj):
                    c += size
                    lasts.append(c - 1)
                lastset = set(lasts)
                for last in lasts:
                    if row127[last] < 0:
                        cand = [jj for jj in range(kj)
                                if row127[jj] >= 0 and jj not in lastset]
                        assert cand, "no non-negative index for lane 127"
                        jj = cand[0]
                        row127[last], row127[jj] = row127[jj], row127[last]
                sel[127] = row127
                # column-major stream, split per gather
                c = 0
                for size in _gather_plan(kj):
                    streams.append(sel[:, c:c + size].T.ravel())
                    c += size
    s = np.concatenate(streams).astype(np.int16)
    wrapped = s.reshape(-1, 16).T
    return np.tile(wrapped, (8, 1)).copy()


def kernel(inputs_pri, inputs_sec, emb_pri, emb_sec, _trace=False, _trace_kwargs=None):
    inputs_pri = np.ascontiguousarray(np.asarray(inputs_pri, dtype=np.int32))
    inputs_sec = np.ascontiguousarray(np.asarray(inputs_sec, dtype=np.int32))
    emb_pri = np.ascontiguousarray(np.asarray(emb_pri, dtype=np.float32))
    emb_sec = np.ascontiguousarray(np.asarray(emb_sec, dtype=np.float32))

    batch = inputs_pri.shape[0]
    bc = batch // N_CORES
    n_blocks = bc // P

    emb_cat = np.zeros((2, N_CHUNKS, CHUNK_ROWS, DIM), np.float32)
    for t, emb in enumerate((emb_pri, emb_sec)):
        for k in range(N_CHUNKS):
            emb_cat[t, k, :CHUNK] = emb[k * CHUNK:(k + 1) * CHUNK]
    emb_cat = np.ascontiguousarray(emb_cat.reshape(2 * N_CHUNKS * CHUNK_ROWS, DIM))

    # Deal rows to cores by global chunk-0-count rank (per table): core c
    # takes rows ranked c, c+8, ... so all cores share near-identical block
    # profiles and the SPMD-shared per-block maxima K carry ~no cross-core
    # padding. Each core's dealt rows are already c0-sorted by construction.
    deal = []     # per core: [2][bc] global row ids, c0-ascending
    K = np.zeros((2, N_CHUNKS, n_blocks), np.int64)
    for t, full in enumerate((inputs_pri, inputs_sec)):
        c0_all = ((full // CHUNK) == 0).sum(axis=1)
        rank = np.argsort(c0_all, kind="stable")
        for c in range(N_CORES):
            ids = rank[c::N_CORES]
            if t == 0:
                deal.append([ids])
            else:
                deal[c].append(ids)
            c0s = c0_all[ids]
            for b in range(n_blocks):
                blk = c0s[b * P:(b + 1) * P]
                K[t, 0, b] = max(K[t, 0, b], blk.max())
                K[t, 1, b] = max(K[t, 1, b], SEQ - blk.min())
    K = np.maximum(K, 1)

    total_cols = int(K.sum())
    idx_cols = total_cols * P // 16
    # first DMA covers just job 0's first piece so gathers start ASAP; the
    # second (bulk) DMA overlaps its generation
    p0 = _gather_plan(int(K[0, 0, 0]))[0]
    split_col = max(64, -(-(p0 * P // 16) // 64) * 64 + 64)
    split_col = min(split_col, idx_cols - 64)

    nc = build_nc(K, n_blocks, idx_cols, split_col)

    in_maps = []
    for c in range(N_CORES):
        rows_c = [inputs_pri[deal[c][0]], inputs_sec[deal[c][1]]]
        gidx = _pack_core(rows_c, K, n_blocks)
        assert gidx.shape[1] == idx_cols
        in_maps.append({"emb_cat": emb_cat, "gidx": gidx})

    kwargs = {}
    if _trace:
        kwargs["trace"] = True
        if _trace_kwargs:
            kwargs.update(_trace_kwargs)
    res = run_bass_kernel_spmd(nc, in_maps, list(range(N_CORES)), **kwargs)
    outs = res.results
    out_pri = np.empty((batch, DIM), np.float32)
    out_sec = np.empty((batch, DIM), np.float32)
    for c in range(N_CORES):
        for t, out_full in enumerate((out_pri, out_sec)):
            res_c = outs[c]["out_pri" if t == 0 else "out_sec"]
            out_full[deal[c][t]] = res_c
    if _trace:
        return (out_pri, out_sec), res
    return out_pri, out_sec



# revision 6
# speedup vs baseline: 7.3069x; 1.0510x over previous
"""Embedding-bag kernel for Trainium2, 8 NeuronCores — v14 (v8 + Q7 warmup overlap).

Design (v4-final + c0-rank dealing):
- Data-parallel: batch rows are dealt to the 8 cores by global chunk-0
  token-count rank (core c takes rows ranked c, c+8, ...), per table, so
  all cores see near-identical per-block column maxima and the SPMD-shared
  K padding collapses (~5% fewer gather indices). Outputs un-permuted on
  the host.
- Both embedding tables replicated per core in DRAM as one concatenated
  tensor (2 tables x 2 vocab chunks of 50001 rows; chunks keep indices in
  int16 range via a +32768 base shift).
- Per (table, 128-row block, chunk): one SWDGE dma_gather job split into
  pieces of <=63 columns (8064 idxs = 505 ring descriptors; 2 fit a
  queue's 1024-desc ring).
- single_packet=False: coalesced packets cap at 64 descriptors (hang
  beyond 8-column gathers); per-descriptor packets also let the 16 SDMA
  engines interleave queues (17.5ns/desc vs 26.3).
- Pieces rotate across all 4 SWDGE queues: the Q7 complex executes one
  gather per queue concurrently (~4-way); per-gather latency is
  ~1.3us + 6.7ns/idx. Single-queue schedules serialize (2.03ms).
- Vector engine reduces each job by contiguous halving adds to <=8
  columns, then one strided tensor_reduce; slot recycling is signalled by
  then_inc on the final reader.

Measured limits (2026-08 session; probes in probe.py/probe2.py, variants in
kernel_v9/v10/v11.py — all slower or equal):
- The bottleneck is SWDGE descriptor GENERATION on the Q7 cpu pairs:
  ~7.8ns/idx per queue (63us per 8064-idx gather), one cpu pair per queue,
  4 queues max (ucode MAX_SWDGE_QUEUES). The 4 pairs generate concurrently
  (the engine instruction parks only while its queue's pair is busy), but
  contention caps the aggregate at ~2.4-2.5ns/idx = ~500-530us for the
  ~219k idxs/core. This kernel sits at that wall; final v14 distribution
  517.4/522.3/523.5/525.5/529.3/547.9us over six runs (~±6% device noise;
  same-period v8 sampled 551-584us).
- DMA drain is NOT the bottleneck: bursts hit ~220GB/s (the 256B-random
  cap; 16 engines x 4 queue rings) while generation feeds ~110GB/s.
- Dead ends measured: prepare_only+trigger_dma serializes generation on
  the engine (1.78ms); transpose-mode gathers generate at the same rate
  (tx cpu still pushes 1 desc/idx); SBUF-source gather 2.58ns/idx (no
  better); single_packet=True at <=8 cols 2.8ns/idx; 32-col pieces equal
  63-col; 8 per-piece slots + ramp (v10) 655us; tail-split (v11) 555us.
- Vector tree reduce ~350us busy under DMA contention (66-142 G elem/s),
  hidden under the gather wall; strided direct tensor_reduce is equal.
- PE/dense-count-matmul hybrids die on building/loading the count matrix
  (dense C is 50-100x the index bytes; bit-packed needs >100us DVE expand).
- Round 2: time budget decomposes as ~428us pure pair generation
  (219k idx x 7.8ns / 4 pairs, pairs ~95% busy in the trace) + ~22us head
  (ucode cold start + gidx DMA) + ~50us tail (last gen+drain, final trees,
  out DMA, ~8us Block teardown). Scheduling variants that targeted the
  residue all landed inside run noise: size-sorted job order (v12, 530us),
  per-piece trees consuming each gather as it lands (v13, 527us),
  tail-split pieces (v11, 555us). This v8 schedule stays.
- Round 3: ap_gather (Q7 SBUF gather, probe3.py) measured ~27ns/idx
  (~221us per 8192-idx call) — 10x worse than dma_gather; not a viable
  alternative path. Ucode libraries load one-at-a-time (PseudoReload-
  LibraryIndex), so dma_gather + ap_gather could not overlap anyway
  without building a custom combined library.
- Round 4 (this version): 4 dummy 128-idx warmup gathers issued before the
  gidx wait overlap the ~10us Q7 ucode cold start per pair with the index
  DMA; first idx chunk shrunk to piece 0 only. Paired same-period runs:
  v14 525.5us vs v8 567.0us.
"""

import sys

if "/opt/trn_rl_repo" not in sys.path:
    sys.path.insert(0, "/opt/trn_rl_repo")

from contextlib import ExitStack

import numpy as np

import concourse.bacc as bacc
import concourse.bass as bass
import concourse.mybir as mybir
from concourse import library_config
from concourse.bass_utils import run_bass_kernel_spmd

N_CORES = 8
P = 128
VOCAB = 100000
SEQ = 200
DIM = 64
BATCH = 4096

N_CHUNKS = 2
CHUNK = VOCAB // N_CHUNKS          # 50000 (signed int16 indexing)
CHUNK_ROWS = CHUNK + 1             # + zero pad row
BASE_SHIFT = 32768                 # in_ap base shifted this many rows in
PAD_IDX = CHUNK - BASE_SHIFT       # local index of the zero row (positive)
GMAX_COLS = 63                     # 8064 idxs -> 505 descs/dma; 2 fit a ring
NBUF = 4
NQ = 4


def _gather_plan(kj):
    """Split kj columns into balanced pieces of <=GMAX_COLS."""
    n = -(-kj // GMAX_COLS)
    base = kj // n
    rem = kj % n
    return [base + (1 if i < rem else 0) for i in range(n)]


def build_nc(K, n_blocks, idx_cols, split_col):
    """K: [2, N_CHUNKS, n_blocks] exact max counts (identical across cores).
    idx_cols: total int16 columns of gidx. split_col: boundary of the first
    idx DMA (gathers whose columns start past it wait for the second DMA)."""
    kmax = int(K.max())

    nc = bacc.Bacc("TRN2", debug=False, num_swdge_queues=NQ)

    emb_cat = nc.dram_tensor(
        "emb_cat", [2 * N_CHUNKS * CHUNK_ROWS, DIM], mybir.dt.float32,
        kind="ExternalInput",
    )
    gidx = nc.dram_tensor("gidx", [P, idx_cols], mybir.dt.int16, kind="ExternalInput")
    out_pri = nc.dram_tensor("out_pri", [n_blocks * P, DIM], mybir.dt.float32, kind="ExternalOutput")
    out_sec = nc.dram_tensor("out_sec", [n_blocks * P, DIM], mybir.dt.float32, kind="ExternalOutput")
    outs = (out_pri, out_sec)

    jobs = [(t, b, k) for t in range(2) for b in range(n_blocks) for k in range(N_CHUNKS)]

    with (
        nc.Block() as _block,
        nc.sbuf_tensor("gidx_sb", [P, idx_cols], mybir.dt.int16) as gidx_sb,
        nc.semaphore("io") as io,
        ExitStack() as stack,
    ):
        slots = [
            stack.enter_context(
                nc.sbuf_tensor(f"slot{i}", [P, kmax * DIM], mybir.dt.float32)
            )
            for i in range(NBUF)
        ]
        accs = [
            stack.enter_context(
                nc.sbuf_tensor(f"acc{t}_{b}", [P, DIM], mybir.dt.float32)
            )
            for t in range(2)
            for b in range(n_blocks)
        ]
        tmp = stack.enter_context(nc.sbuf_tensor("tmp", [P, DIM], mybir.dt.float32))
        done = [
            [stack.enter_context(nc.semaphore(f"done{i}_{q}")) for q in range(NQ)]
            for i in range(NBUF)
        ]
        free = [stack.enter_context(nc.semaphore(f"free{i}")) for i in range(NBUF)]
        oready = stack.enter_context(nc.semaphore("oready"))

        # ---- sync engine: two-stage index load so gathers start early
        nc.sync.dma_start(gidx_sb[:, :split_col], gidx[:, :split_col]).then_inc(io, 16)
        nc.sync.dma_start(gidx_sb[:, split_col:], gidx[:, split_col:]).then_inc(io, 16)

        # ---- gpsimd: warm all 4 Q7 pairs (ucode cold start ~10us) while the
        # first gidx chunk is still in flight: 128 dummy idxs (zeros) per queue
        # into a scratch slot. Their DMAs complete in a few us; nothing waits.
        nc.gpsimd.load_library(library_config.mlp)
        widx = stack.enter_context(nc.sbuf_tensor("widx", [P, 8], mybir.dt.int16))
        wslot = stack.enter_context(nc.sbuf_tensor("wslot", [P, 8 * DIM], mybir.dt.float32))
        warm = stack.enter_context(nc.semaphore("warm"))
        nc.gpsimd.memset(widx[:], 0)
        w3 = wslot[:].rearrange("p (c d) -> p c d", d=DIM)
        wsrc = emb_cat[BASE_SHIFT:CHUNK_ROWS, :]
        for q in range(NQ):
            nc.gpsimd.dma_gather(
                w3[:, :1, :], wsrc, widx[:], P, P, DIM,
                queue_num=q, single_packet=False,
            ).then_inc(warm, 16)
        nc.gpsimd.wait_ge(io, 16)
        waited_full = False
        gq = 0            # queue rotation counter
        icol = 0          # running int16 column offset into gidx_sb
        done_target = [[0] * NQ for _ in range(NBUF)]
        for j, (t, b, k) in enumerate(jobs):
            slot = j % NBUF
            if j >= NBUF:
                nc.gpsimd.wait_ge(free[slot], j // NBUF)
            kj = int(K[t, k, b])
            base = (t * N_CHUNKS + k) * CHUNK_ROWS + BASE_SHIFT
            src = emb_cat[base:(t * N_CHUNKS + k + 1) * CHUNK_ROWS, :]
            g3 = slots[slot][:].rearrange("p (c d) -> p c d", d=DIM)
            col = 0
            for size in _gather_plan(kj):
                nidx = size * P
                ic = nidx // 16
                if not waited_full and icol + ic > split_col:
                    nc.gpsimd.wait_ge(io, 32)
                    waited_full = True
                q = gq % NQ
                nc.gpsimd.dma_gather(
                    g3[:, col:col + size, :],
                    src,
                    gidx_sb[:, icol:icol + ic],
                    nidx,
                    nidx,
                    DIM,
                    queue_num=q,
                    single_packet=False,
                ).then_inc(done[slot][q], 16)
                done_target[slot][q] += 16
                gq += 1
                icol += ic
                col += size
            jobs[j] = (t, b, k, slot, tuple(done_target[slot]), kj)

        # ---- vector: halving-tree reduce, accumulate chunks, recycle slots
        for j, (t, b, k, slot, tgts, kj) in enumerate(jobs):
            for q in range(NQ):
                if tgts[q]:
                    nc.vector.wait_ge(done[slot][q], tgts[q])
            g = slots[slot]
            n = kj
            while n > 8:
                h = n // 2
                nc.vector.tensor_add(
                    out=g[:, : h * DIM],
                    in0=g[:, : h * DIM],
                    in1=g[:, (n - h) * DIM : n * DIM],
                )
                n -= h
            gv = g[:].rearrange("p (c d) -> p d c", d=DIM)[:, :, :n]
            acc = accs[t * n_blocks + b]
            red_out = acc if k == 0 else tmp
            red = nc.vector.tensor_reduce(
                out=red_out[:], in_=gv, axis=mybir.AxisListType.X,
                op=mybir.AluOpType.add,
            )
            # the reduce is the last reader of the slot
            red.then_inc(free[slot], 1)
            if k == 0:
                continue
            nc.vector.tensor_add(out=acc[:], in0=acc[:], in1=tmp[:]).then_inc(
                oready, 1
            )

        # ---- sync engine: write outputs as accs complete
        m = 0
        for t in range(2):
            for b in range(n_blocks):
                m += 1
                nc.sync.wait_ge(oready, m)
                nc.sync.dma_start(
                    out=outs[t][b * P:(b + 1) * P, :],
                    in_=accs[t * n_blocks + b][:],
                ).then_inc(io, 16)
        nc.sync.wait_ge(io, 32 + m * 16)

    nc.compile()
    return nc


def _pack_core(idx_sorted, K, n_blocks):
    """idx_sorted: [2][bc, SEQ] row-sorted core indices. Returns gidx."""
    streams = []
    for t in range(2):
        for b in range(n_blocks):
            rows = idx_sorted[t][b * P:(b + 1) * P]
            for k in range(N_CHUNKS):
                kj = int(K[t, k, b])
                mask = (rows // CHUNK) == k
                local = (rows - k * CHUNK - BASE_SHIFT).astype(np.int64)
                order = np.argsort(~mask, axis=1, kind="stable")
                sortloc = np.take_along_axis(local, order, axis=1)
                cnt = mask.sum(axis=1)
                pad_cols = max(kj - SEQ, 0)
                if pad_cols:
                    sortloc = np.concatenate(
                        [sortloc, np.zeros((P, pad_cols), np.int64)], axis=1
                    )
                sel = sortloc[:, :kj]
                sel = np.where(np.arange(kj)[None, :] < cnt[:, None], sel, PAD_IDX)
                # Every gather's final stream slot (lane 127, last column of
                # the gather) must be >= 0: ucode trims trailing negatives.
                row127 = sel[127].copy()
                lasts = []
                c = 0
                for size in _gather_plan(kj):
                    c += size
                    lasts.append(c - 1)
                lastset = set(lasts)
                for last in lasts:
                    if row127[last] < 0:
                        cand = [jj for jj in range(kj)
                                if row127[jj] >= 0 and jj not in lastset]
                        assert cand, "no non-negative index for lane 127"
                        jj = cand[0]
                        row127[last], row127[jj] = row127[jj], row127[last]
                sel[127] = row127
                # column-major stream, split per gather
                c = 0
                for size in _gather_plan(kj):
                    streams.append(sel[:, c:c + size].T.ravel())
                    c += size
    s = np.concatenate(streams).astype(np.int16)
    wrapped = s.reshape(-1, 16).T
    return np.tile(wrapped, (8, 1)).copy()


def kernel(inputs_pri, inputs_sec, emb_pri, emb_sec, _trace=False, _trace_kwargs=None):
    inputs_pri = np.ascontiguousarray(np.asarray(inputs_pri, dtype=np.int32))
    inputs_sec = np.ascontiguousarray(np.asarray(inputs_sec, dtype=np.int32))
    emb_pri = np.ascontiguousarray(np.asarray(emb_pri, dtype=np.float32))
    emb_sec = np.ascontiguousarray(np.asarray(emb_sec, dtype=np.float32))

    batch = inputs_pri.shape[0]
    bc = batch // N_CORES
    n_blocks = bc // P

    emb_cat = np.zeros((2, N_CHUNKS, CHUNK_ROWS, DIM), np.float32)
    for t, emb in enumerate((emb_pri, emb_sec)):
        for k in range(N_CHUNKS):
            emb_cat[t, k, :CHUNK] = emb[k * CHUNK:(k + 1) * CHUNK]
    emb_cat = np.ascontiguousarray(emb_cat.reshape(2 * N_CHUNKS * CHUNK_ROWS, DIM))

    # Deal rows to cores by global chunk-0-count rank (per table): core c
    # takes rows ranked c, c+8, ... so all cores share near-identical block
    # profiles and the SPMD-shared per-block maxima K carry ~no cross-core
    # padding. Each core's dealt rows are already c0-sorted by construction.
    deal = []     # per core: [2][bc] global row ids, c0-ascending
    K = np.zeros((2, N_CHUNKS, n_blocks), np.int64)
    for t, full in enumerate((inputs_pri, inputs_sec)):
        c0_all = ((full // CHUNK) == 0).sum(axis=1)
        rank = np.argsort(c0_all, kind="stable")
        for c in range(N_CORES):
            ids = rank[c::N_CORES]
            if t == 0:
                deal.append([ids])
            else:
                deal[c].append(ids)
            c0s = c0_all[ids]
            for b in range(n_blocks):
                blk = c0s[b * P:(b + 1) * P]
                K[t, 0, b] = max(K[t, 0, b], blk.max())
                K[t, 1, b] = max(K[t, 1, b], SEQ - blk.min())
    K = np.maximum(K, 1)

    total_cols = int(K.sum())
    idx_cols = total_cols * P // 16
    # first DMA covers just job 0's first piece so gathers start ASAP; the
    # second (bulk) DMA overlaps its generation
    p0 = _gather_plan(int(K[0, 0, 0]))[0]
    split_col = max(64, -(-(p0 * P // 16) // 64) * 64 + 64)
    split_col = min(split_col, idx_cols - 64)

    nc = build_nc(K, n_blocks, idx_cols, split_col)

    in_maps = []
    for c in range(N_CORES):
        rows_c = [inputs_pri[deal[c][0]], inputs_sec[deal[c][1]]]
        gidx = _pack_core(rows_c, K, n_blocks)
        assert gidx.shape[1] == idx_cols
        in_maps.append({"emb_cat": emb_cat, "gidx": gidx})

    kwargs = {}
    if _trace:
        kwargs["trace"] = True
        if _trace_kwargs:
            kwargs.update(_trace_kwargs)
    res = run_bass_kernel_spmd(nc, in_maps, list(range(N_CORES)), **kwargs)
    outs = res.results
    out_pri = np.empty((batch, DIM), np.float32)
    out_sec = np.empty((batch, DIM), np.float32)
    for c in range(N_CORES):
        for t, out_full in enumerate((out_pri, out_sec)):
            res_c = outs[c]["out_pri" if t == 0 else "out_sec"]
            out_full[deal[c][t]] = res_c
    if _trace:
        return (out_pri, out_sec), res
    return out_pri, out_sec

